# revision 8
# baseline (speedup 1.0000x reference)
"""NeuralBlendshapes Trainium2 kernel (8-core SPMD Bass/Tile).

Sharding:
  - ed MLP (B*F rows, ~52 GFLOP): faces sharded 8 ways, feature-major matmuls.
  - Poisson RHS: per-core dma_scatter_add (3 corners x collision-free rounds)
    into [11264, 64] DRAM, AllReduce across cores.
  - L_inv solve: row-sharded; each core streams its transposed [11248+pad, 1408]
    chunk as the matmul moving operand; back-region column-mean folded in as an
    extra column.
  - td/pw/inn MLPs + apply_def epilogue: vertex-chunk sharded (1406 head + 352
    innards rows per core).
Host (numpy): positional encodings, deformed_ict einsum, ict_jac, index/round
tables, L_inv chunk transposes, output assembly.
"""
import sys
import os

sys.path.insert(0, "/opt/trn_rl_repo")

import numpy as np
from contextlib import ExitStack

import concourse.bass as bass
import concourse.bacc as bacc
import concourse.tile as tile
import concourse.mybir as mybir
from concourse.masks import make_identity

AluOp = mybir.AluOpType
ActFn = mybir.ActivationFunctionType
dt = mybir.dt

HEAD = 11248
V = 14062
F = 22000
B = 4
NCORES = 8

FCH = F // NCORES            # 2750 faces per core
FPAD = 2816                  # 22 f-blocks of 128
TF = FPAD // 128             # 22
NED = FPAD * 4               # 11264 ed-MLP columns (rho = f*4 + b)
HCH = HEAD // NCORES         # 1406 head rows per core
HP = 1408                    # head rows padded (11 tiles)
VI = V - HEAD                # 2814 innards
ICH = 352                    # innards rows per core (last core 350)
IP = 384                     # innards rows padded (3 tiles)
OWNP = HP + IP               # 1792 own rows padded
NTILE = OWNP // 128          # 14 row tiles
NHT = HP // 128              # 11 head tiles
NRR = 11264                  # rhs rows (88 chunks of 128)
NJ = NRR // 128              # 88
ESTEP = 64                   # rhs row stride (256B)
DUMP = NRR - 1               # dump row for padded/invalid scatter slots
IW = FPAD // 16              # 176 idx columns
NIN = IP * 4                 # 1536 inn-MLP columns

_CACHE = {}


def _embed(x):
    freqs = (2.0 ** np.arange(4)).astype(np.float32)
    xf = x[..., None] * freqs
    s = np.sin(xf).reshape(*x.shape[:-1], -1)
    c = np.cos(xf).reshape(*x.shape[:-1], -1)
    return np.concatenate([x, s, c], -1).astype(np.float32)


def _occurrence_rank(a):
    """occ[i] = number of earlier elements equal to a[i]."""
    order = np.argsort(a, kind="stable")
    sa = a[order]
    grp_start = np.r_[0, np.nonzero(np.diff(sa))[0] + 1]
    starts = np.zeros(len(a), np.int64)
    starts[grp_start] = grp_start
    starts = np.maximum.accumulate(starts)
    occ_sorted = np.arange(len(a)) - starts
    occ = np.empty(len(a), np.int64)
    occ[order] = occ_sorted
    return occ


def _wrap16(idx_flat):
    """[N] -> [128, N/16] int16, o = s*16 + p, replicated to 128 partitions."""
    n = len(idx_flat)
    assert n % 16 == 0
    i16 = idx_flat.reshape(n // 16, 16).T.astype(np.int16)
    return np.tile(i16, (8, 1))


def _mk_ap(t, offset, dims):
    return bass.AP(t.tensor if hasattr(t, "tensor") else t, offset, [list(d) for d in dims])


def _sb(tile_obj, free_dims, extra_off=0):
    """AP over an SBUF tile with custom free dims (partition dim kept)."""
    a = tile_obj[:]
    return bass.AP(a.tensor, a.offset + extra_off, [list(a.ap[0])] + [list(d) for d in free_dims])


def _build(rounds_per_corner):
    """Build the SPMD Bacc program. rounds_per_corner: tuple of 3 ints."""
    nc = bacc.Bacc("TRN2", target_bir_lowering=False, debug=False, num_devices=NCORES)
    NROUND = sum(rounds_per_corner)

    # ---------------- DRAM I/O ----------------
    def din(name, shape, dd=dt.float32):
        return nc.dram_tensor(name, shape, dd, kind="ExternalInput")

    def dout(name, shape, dd=dt.float32):
        return nc.dram_tensor(name, shape, dd, kind="ExternalOutput")

    xe_d = din("xe", [116, NED])
    icj_d = din("icj", [128, TF, 36])
    cwp_d = din("cwp", [128, TF, 9])
    sidx_d = din("sidx", [128, NROUND, IW], dt.int16)
    lit_d = din("lit", [NRR, HP])
    xtd_d = din("xtd", [27, OWNP])
    xpw_d = din("xpw", [3, OWNP])
    xin_d = din("xin", [83, NIN])
    dicto_d = din("dicto", [OWNP, 12])
    dictb_d = din("dictb", [2816, 12])
    feat_d = din("feat", [128, 64])
    codet_d = din("codet", [53, 4])
    ew_in_d = din("ew_in", [116, 256])
    ew_h_d = din("ew_h", [4, 256, 256])
    ew_out_d = din("ew_out", [256, 9])
    eb_in_d = din("eb_in", [256])
    eb_h_d = din("eb_h", [4, 256])
    eb_out_d = din("eb_out", [9])
    tw_in_d = din("tw_in", [27, 256])
    tw_h_d = din("tw_h", [4, 256, 256])
    tw_out_d = din("tw_out", [256, 3])
    tb_in_d = din("tb_in", [256])
    tb_h_d = din("tb_h", [4, 256])
    tb_out_d = din("tb_out", [3])
    iw_in_d = din("iw_in", [83, 256])
    iw_h_d = din("iw_h", [4, 256, 256])
    iw_out_d = din("iw_out", [256, 3])
    ib_in_d = din("ib_in", [256])
    ib_h_d = din("ib_h", [4, 256])
    ib_out_d = din("ib_out", [3])
    gw_in_d = din("gw_in", [53, 32])
    gw_h_d = din("gw_h", [32, 32])
    gw_out_d = din("gw_out", [32, 3])
    gb_in_d = din("gb_in", [32])
    gb_h_d = din("gb_h", [32])
    gb_out_d = din("gb_out", [3])
    pw_in_d = din("pw_in", [3, 32])
    pw_h_d = din("pw_h", [32, 32])
    pw_out_d = din("pw_out", [32, 1])
    pb_in_d = din("pb_in", [32])
    pb_h_d = din("pb_h", [32])
    pb_out_d = din("pb_out", [1])
    scal_d = din("scal", [128, 16])  # baked gauss scales etc, row-replicated

    tdef_o = dout("tdef_o", [OWNP, 3])
    pw_o = dout("pw_o", [OWNP, 1])
    ev_o = dout("ev_o", [OWNP, 12])
    df_o = dout("df_o", [OWNP, 12])
    oi_o = dout("oi_o", [OWNP, 12])

    rhs_d = nc.dram_tensor("rhs_loc", [NRR, ESTEP], dt.float32)
    rhsr_d = nc.dram_tensor("rhs_red", [NRR, ESTEP], dt.float32, addr_space="Shared")

    # scale slot indices in scal_d (values baked host-side):
    # 0..4: ed layer scales; 5..9: td; 10..14: inn  -> passed as imm floats instead
    # (scal_d kept for potential future use; scales are compile-time floats here)

    with tile.TileContext(nc) as tc, ExitStack() as ctx:
        wp = ctx.enter_context(tc.tile_pool(name="wp", bufs=1))
        hp_pool = ctx.enter_context(tc.tile_pool(name="hp", bufs=2))
        ap_pool = ctx.enter_context(tc.tile_pool(name="apl", bufs=4))
        ep_pool = ctx.enter_context(tc.tile_pool(name="epl", bufs=3))
        lit_pool = ctx.enter_context(tc.tile_pool(name="litp", bufs=4))
        pp = ctx.enter_context(tc.tile_pool(name="pp", bufs=4, space="PSUM"))
        pl = ctx.enter_context(tc.tile_pool(name="pl", bufs=1, space="PSUM"))

        sync = nc.sync

        # ---------------- resident loads ----------------
        icj_t = wp.tile([128, TF, 36], dt.float32)
        sync.dma_start(icj_t[:], icj_d.ap())
        cwp_t = wp.tile([128, TF, 9], dt.float32)
        sync.dma_start(cwp_t[:], cwp_d.ap())
        sidx_t = wp.tile([128, NROUND, IW], dt.int16)
        sync.dma_start(sidx_t[:], sidx_d.ap())
        dicto_t = wp.tile([128, NTILE, 12], dt.float32)
        sync.dma_start(dicto_t[:], _mk_ap(dicto_d.ap(), 0, [[12, 128], [12 * 128, NTILE], [1, 12]]))
        dictb_t = wp.tile([128, 22, 12], dt.float32)
        sync.dma_start(dictb_t[:], _mk_ap(dictb_d.ap(), 0, [[12, 128], [12 * 128, 22], [1, 12]]))
        feat_t = wp.tile([128, 64], dt.float32)
        sync.dma_start(feat_t[:], feat_d.ap())
        codet_t = wp.tile([53, 4], dt.float32)
        sync.dma_start(codet_t[:], codet_d.ap())
        xtd_t = wp.tile([27, OWNP], dt.float32)
        sync.dma_start(xtd_t[:], xtd_d.ap())
        xpw_t = wp.tile([3, OWNP], dt.float32)
        sync.dma_start(xpw_t[:], xpw_d.ap())
        xin_t = wp.tile([83, NIN], dt.float32)
        sync.dma_start(xin_t[:], xin_d.ap())

        def load_w(dram, shape, ap_dims):
            t = wp.tile(shape, dt.float32, name=f"w_{dram.name}")
            sync.dma_start(t[:], _mk_ap(dram.ap(), 0, ap_dims))
            return t

        ew_in = load_w(ew_in_d, [116, 256], [[256, 116], [1, 256]])
        ew_h = load_w(ew_h_d, [128, 8, 256], [[256, 128], [65536, 4], [32768, 2], [1, 256]])
        ew_out = load_w(ew_out_d, [128, 2, 9], [[9, 128], [9 * 128, 2], [1, 9]])
        eb_in = load_w(eb_in_d, [128, 2], [[1, 128], [128, 2]])
        eb_h = load_w(eb_h_d, [128, 8], [[1, 128], [256, 4], [128, 2]])
        tw_in = load_w(tw_in_d, [27, 256], [[256, 27], [1, 256]])
        tw_h = load_w(tw_h_d, [128, 8, 256], [[256, 128], [65536, 4], [32768, 2], [1, 256]])
        tw_out = load_w(tw_out_d, [128, 2, 3], [[3, 128], [3 * 128, 2], [1, 3]])
        tb_in = load_w(tb_in_d, [128, 2], [[1, 128], [128, 2]])
        tb_h = load_w(tb_h_d, [128, 8], [[1, 128], [256, 4], [128, 2]])
        iw_in = load_w(iw_in_d, [83, 256], [[256, 83], [1, 256]])
        iw_h = load_w(iw_h_d, [128, 8, 256], [[256, 128], [65536, 4], [32768, 2], [1, 256]])
        iw_out = load_w(iw_out_d, [128, 2, 3], [[3, 128], [3 * 128, 2], [1, 3]])
        ib_in = load_w(ib_in_d, [128, 2], [[1, 128], [128, 2]])
        ib_h = load_w(ib_h_d, [128, 8], [[1, 128], [256, 4], [128, 2]])
        gw_in = load_w(gw_in_d, [53, 32], [[32, 53], [1, 32]])
        gw_h = load_w(gw_h_d, [32, 32], [[32, 32], [1, 32]])
        gw_out = load_w(gw_out_d, [32, 3], [[3, 32], [1, 3]])
        gb_in = load_w(gb_in_d, [32, 1], [[1, 32], [1, 1]])
        gb_h = load_w(gb_h_d, [32, 1], [[1, 32], [1, 1]])
        pw_in = load_w(pw_in_d, [3, 32], [[32, 3], [1, 32]])
        pw_h = load_w(pw_h_d, [32, 32], [[32, 32], [1, 32]])
        pw_out = load_w(pw_out_d, [32, 1], [[1, 32], [1, 1]])
        pb_in = load_w(pb_in_d, [32, 1], [[1, 32], [1, 1]])
        pb_h = load_w(pb_h_d, [32, 1], [[1, 32], [1, 1]])

        ident = wp.tile([128, 128], dt.float32)
        make_identity(nc, ident[:])
        ones_t = wp.tile([128, 1], dt.float32)
        nc.vector.memset(ones_t[:], 1.0)

        # gauss scales baked as python floats via closure (set at build call)
        ed_s, td_s, inn_s, gt_s, pw_s = _build.scales

        # biases for out layers are tiny; add via const tiles (partition-broadcast DMA)
        ebo_t = wp.tile([128, 9], dt.float32)
        sync.dma_start(ebo_t[:], _mk_ap(eb_out_d.ap(), 0, [[0, 128], [1, 9]]))
        tbo_t = wp.tile([128, 3], dt.float32)
        sync.dma_start(tbo_t[:], _mk_ap(tb_out_d.ap(), 0, [[0, 128], [1, 3]]))
        ibo_t = wp.tile([128, 3], dt.float32)
        sync.dma_start(ibo_t[:], _mk_ap(ib_out_d.ap(), 0, [[0, 128], [1, 3]]))
        gbo_t = wp.tile([4, 3], dt.float32)
        sync.dma_start(gbo_t[:], _mk_ap(gb_out_d.ap(), 0, [[0, 4], [1, 3]]))
        pbo_t = wp.tile([128, 1], dt.float32)
        sync.dma_start(pbo_t[:], _mk_ap(pb_out_d.ap(), 0, [[0, 128], [1, 1]]))

        # ---------------- generic feature-major MLP hidden stack ----------------
        def act_block(psum_ap, bias_ap, scale, out_ap):
            """out = exp(scale * (psum + bias)^2); bias [P,1]."""
            p = psum_ap.ap[0][1]
            n = psum_ap.ap[-1][1]
            t1 = ap_pool.tile([128, 512], dt.float32, tag="act1")
            nc.vector.tensor_scalar_add(t1[0:p, 0:n], psum_ap, bias_ap)
            t2 = ap_pool.tile([128, 512], dt.float32, tag="act2")
            nc.vector.tensor_mul(t2[0:p, 0:n], t1[0:p, 0:n], t1[0:p, 0:n])
            nc.scalar.activation(out_ap, t2[0:p, 0:n], ActFn.Exp, bias=0.0, scale=scale)

        def hidden_stack(x_ap, ncols, w_in, b_in, w_h, b_h, scales, htag):
            """x [kin, ncols] -> h4 tile [128, 2, ncols] (256 features)."""
            nts = [(i * 512, min(512, ncols - i * 512)) for i in range((ncols + 511) // 512)]
            h = hp_pool.tile([128, 2, ncols], dt.float32, tag=htag)
            for fo in range(2):
                for base, n in nts:
                    psm = pp.tile([128, 512], dt.float32, tag="mm")
                    nc.tensor.matmul(
                        psm[:, 0:n],
                        _mk_ap(w_in[:].tensor, w_in[:].offset + fo * 128, [list(w_in[:].ap[0]), [1, 128]]),
                        bass.AP(x_ap.tensor, x_ap.offset + base, [list(x_ap.ap[0]), [1, n]]),
                        start=True, stop=True,
                    )
                    act_block(psm[:, 0:n], b_in[:, fo:fo + 1], scales[0],
                              _sb(h, [[1, n]], fo * ncols + base))
            for li in range(4):
                h2 = hp_pool.tile([128, 2, ncols], dt.float32, tag=htag)
                for fo in range(2):
                    for base, n in nts:
                        psm = pp.tile([128, 512], dt.float32, tag="mm")
                        for k in range(2):
                            nc.tensor.matmul(
                                psm[:, 0:n],
                                _sb(w_h, [[1, 128]], (li * 2 + k) * 256 + fo * 128),
                                _sb(h, [[1, n]], k * ncols + base),
                                start=(k == 0), stop=(k == 1),
                            )
                        act_block(psm[:, 0:n], b_h[:, li * 2 + fo:li * 2 + fo + 1], scales[1 + li],
                                  _sb(h2, [[1, n]], fo * ncols + base))
                h = h2
            return h

        # ---------------- ed MLP + contrib + scatter ----------------
        src_slab = wp.tile([128, 3, TF, 12], dt.float32)

        ed_blocks = [(0, 2048), (2048, 2048), (4096, 2048), (6144, 2048), (8192, 2048), (10240, 1024)]
        for cb_base, cb_n in ed_blocks:
            xcb = hp_pool.tile([116, 2048], dt.float32, tag="xcb")
            sync.dma_start(xcb[:, 0:cb_n], _mk_ap(xe_d.ap(), cb_base, [[NED, 116], [1, cb_n]]))
            x_ap = bass.AP(xcb[:].tensor, xcb[:].offset, [list(xcb[:].ap[0]), [1, cb_n]])
            h4 = hidden_stack(x_ap, cb_n, ew_in, eb_in, ew_h, eb_h, ed_s, "h")
            # J' layer per f-block (512 cols each)
            for fb in range(cb_n // 512):
                t_glob = cb_base // 512 + fb
                psj = pp.tile([128, 512], dt.float32, tag="mm")
                for b in range(B):
                    for k in range(2):
                        nc.tensor.matmul(
                            psj[:, b * 9:(b + 1) * 9],
                            _sb(h4, [[4, 128]], k * cb_n + fb * 512 + b),
                            _sb(ew_out, [[1, 9]], k * 9),
                            start=(k == 0), stop=(k == 1),
                        )
                # J' = psum + bias_out + ict_jac  (bias broadcast over (b))
                jp = ep_pool.tile([128, 36], dt.float32, tag="jp")
                nc.vector.tensor_tensor(
                    jp[:], psj[:, 0:36],
                    _sb(icj_t, [[1, 36]], t_glob * 36),
                    op=AluOp.add)
                nc.vector.tensor_tensor(
                    jp[:], jp[:],
                    _sb(ebo_t, [[0, 4], [1, 9]]),
                    op=AluOp.add)
                # contrib: src_slab[:, c, t, :] = sum_i jp[(b, i, jj)] * cw[(c, i)]
                tmp = ep_pool.tile([128, 36], dt.float32, tag="ctmp")
                for i in range(3):
                    in0 = _sb(jp, [[0, 3], [9, 4], [1, 3]], i * 3)
                    in1 = _sb(cwp_t, [[3, 3], [0, 4], [0, 3]], t_glob * 9 + i)
                    if i == 0:
                        nc.vector.tensor_tensor(
                            _sb(src_slab, [[TF * 12, 3], [3, 4], [1, 3]], t_glob * 12),
                            in0, in1, op=AluOp.mult)
                    else:
                        nc.vector.tensor_tensor(
                            _sb(tmp, [[12, 3], [3, 4], [1, 3]]), in0, in1, op=AluOp.mult)
                        nc.vector.tensor_tensor(
                            _sb(src_slab, [[TF * 12, 3], [3, 4], [1, 3]], t_glob * 12),
                            _sb(src_slab, [[TF * 12, 3], [3, 4], [1, 3]], t_glob * 12),
                            _sb(tmp, [[12, 3], [3, 4], [1, 3]]),
                            op=AluOp.add)

        # zero rhs then scatter-add rounds
        z_t = wp.tile([128, ESTEP], dt.float32)
        nc.vector.memset(z_t[:], 0.0)
        oa = rhs_d.ap()
        sync.dma_start(
            _mk_ap(oa, 0, [[ESTEP * NJ, 128], [ESTEP, NJ], [1, ESTEP]]),
            _sb(z_t, [[0, NJ], [1, ESTEP]]),
        )
        ridx = 0
        for c in range(3):
            for r in range(rounds_per_corner[c]):
                nc.gpsimd.dma_scatter_add(
                    out_ap=_mk_ap(oa, 0, [[ESTEP, NRR], [1, 12]]),
                    in_ap=src_slab[:, c, :, :],
                    idxs_ap=sidx_t[:, ridx, :],
                    num_idxs=FPAD,
                    num_idxs_reg=FPAD,
                    elem_size=12,
                    elem_step=ESTEP,
                )
                ridx += 1

        nc.gpsimd.collective_compute(
            "AllReduce", AluOp.add,
            replica_groups=[list(range(NCORES))],
            ins=[rhs_d.ap()], outs=[rhsr_d.ap()],
        )

        rhs_sb = wp.tile([128, NJ, ESTEP], dt.float32)
        sync.dma_start(rhs_sb[:], _mk_ap(rhsr_d.ap(), 0, [[ESTEP, 128], [ESTEP * 128, NJ], [1, ESTEP]]))

        # ---------------- L_inv chunk matmul ----------------
        lps = [pl.tile([12, 512], dt.float32, name=f"lp{i}") for i in range(3)]
        vws = [512, 512, 384]
        for j in range(NJ):
            slab = lit_pool.tile([128, HP], dt.float32, tag="slab")
            sync.dma_start(slab[:], _mk_ap(lit_d.ap(), j * 128 * HP, [[HP, 128], [1, HP]]))
            for vt in range(3):
                nc.tensor.matmul(
                    lps[vt][:, 0:vws[vt]],
                    rhs_sb[:, j, 0:12],
                    slab[:, vt * 512: vt * 512 + vws[vt]],
                    start=(j == 0), stop=(j == NJ - 1),
                )
        expr_sb = wp.tile([12, HP], dt.float32)
        for vt in range(3):
            nc.vector.tensor_copy(expr_sb[:, vt * 512: vt * 512 + vws[vt]], lps[vt][:, 0:vws[vt]])

        # ---------------- td / pw / inn MLPs ----------------
        h4t = hidden_stack(xtd_t[:], OWNP, tw_in, tb_in, tw_h, tb_h, td_s, "h")
        tdef_all = wp.tile([128, NTILE, 3], dt.float32)
        for t in range(NTILE):
            pst = pp.tile([128, 512], dt.float32, tag="mm")
            for k in range(2):
                nc.tensor.matmul(
                    pst[:, 0:3],
                    _sb(h4t, [[1, 128]], k * OWNP + t * 128),
                    _sb(tw_out, [[1, 3]], k * 3),
                    start=(k == 0), stop=(k == 1),
                )
            nc.vector.tensor_tensor(tdef_all[:, t, :], pst[:, 0:3], tbo_t[:, 0:3], op=AluOp.add)
            sync.dma_start(_mk_ap(tdef_o.ap(), t * 128 * 3, [[3, 128], [1, 3]]), tdef_all[:, t, :])

        # pw MLP (dh=32, 1 hidden layer) + sigmoid
        psp = pp.tile([128, 512], dt.float32, tag="mm")
        hpw = wp.tile([32, OWNP], dt.float32)
        for base, n in [(0, 512), (512, 512), (1024, 512), (1536, 256)]:
            psm = pp.tile([128, 512], dt.float32, tag="mm")
            nc.tensor.matmul(psm[0:32, 0:n], pw_in[:],
                             bass.AP(xpw_t[:].tensor, xpw_t[:].offset + base, [list(xpw_t[:].ap[0]), [1, n]]),
                             start=True, stop=True)
            act_block(psm[0:32, 0:n], pb_in[0:32, :], pw_s[0], hpw[:, base:base + n])
        hpw2 = wp.tile([32, OWNP], dt.float32)
        for base, n in [(0, 512), (512, 512), (1024, 512), (1536, 256)]:
            psm = pp.tile([128, 512], dt.float32, tag="mm")
            nc.tensor.matmul(psm[0:32, 0:n], pw_h[:], hpw[:, base:base + n], start=True, stop=True)
            act_block(psm[0:32, 0:n], pb_h[0:32, :], pw_s[1], hpw2[:, base:base + n])
        pw_all = wp.tile([128, NTILE, 1], dt.float32)
        for t in range(NTILE):
            psm = pp.tile([128, 512], dt.float32, tag="mm")
            nc.tensor.matmul(psm[:, 0:1], hpw2[:, t * 128:(t + 1) * 128], pw_out[0:32, :],
                             start=True, stop=True)
            tt = ep_pool.tile([128, 1], dt.float32, tag="pwt")
            nc.vector.tensor_tensor(tt[:], psm[:, 0:1], pbo_t[:], op=AluOp.add)
            th = ep_pool.tile([128, 1], dt.float32, tag="pwh")
            nc.scalar.activation(th[:], tt[:], ActFn.Tanh, bias=0.0, scale=0.5)
            nc.vector.tensor_scalar(pw_all[:, t, :], th[:], 0.5, 0.5, op0=AluOp.mult, op1=AluOp.add)
            sync.dma_start(_mk_ap(pw_o.ap(), t * 128, [[1, 128], [1, 1]]), pw_all[:, t, :])

        # inn MLP
        h4i = hidden_stack(xin_t[:], NIN, iw_in, ib_in, iw_h, ib_h, inn_s, "h")
        inn_all = wp.tile([128, 3, 12], dt.float32)
        for t in range(3):
            psi = pp.tile([128, 512], dt.float32, tag="mm")
            for b in range(B):
                for k in range(2):
                    nc.tensor.matmul(
                        psi[:, b * 3:(b + 1) * 3],
                        _sb(h4i, [[4, 128]], k * NIN + t * 512 + b),
                        _sb(iw_out, [[1, 3]], k * 3),
                        start=(k == 0), stop=(k == 1),
                    )
            nc.vector.tensor_tensor(inn_all[:, t, :], psi[:, 0:12],
                                    _sb(ibo_t, [[0, 4], [1, 3]]), op=AluOp.add)

        # ---------------- gt MLP + g ----------------
        psg = pp.tile([128, 512], dt.float32, tag="mm")
        nc.tensor.matmul(psg[0:32, 0:4], gw_in[:], codet_t[:], start=True, stop=True)
        hg = wp.tile([32, 4], dt.float32)
        act_block(psg[0:32, 0:4], gb_in[0:32, :], gt_s[0], hg[:])
        psg2 = pp.tile([128, 512], dt.float32, tag="mm")
        nc.tensor.matmul(psg2[0:32, 0:4], gw_h[:], hg[:], start=True, stop=True)
        hg2 = wp.tile([32, 4], dt.float32)
        act_block(psg2[0:32, 0:4], gb_h[0:32, :], gt_s[1], hg2[:])
        psg3 = pp.tile([128, 512], dt.float32, tag="mm")
        nc.tensor.matmul(psg3[0:4, 0:3], hg2[:], gw_out[:], start=True, stop=True)
        gt_sb = wp.tile([4, 3], dt.float32)
        nc.vector.tensor_tensor(gt_sb[:], psg3[0:4, 0:3], gbo_t[:], op=AluOp.add)
        gt_col = wp.tile([12, 1], dt.float32)
        sync.dma_start(gt_col[:], gt_sb[:])

        # mean of deformed_ict back rows: [12, 1] via ones-matmul
        psb = pl.tile([12, 512], dt.float32, name="lpb")
        for t in range(22):
            nc.tensor.matmul(psb[:, 0:1], dictb_t[:, t, :], ones_t[:],
                             start=(t == 0), stop=(t == 21))
        g_col = wp.tile([12, 1], dt.float32)
        # g = mean_back_ict - m.rhs + gt
        nc.vector.tensor_scalar(g_col[:], psb[:, 0:1], 1.0 / 2704.0, 0.0, op0=AluOp.mult, op1=AluOp.add)
        nc.vector.tensor_sub(g_col[:], g_col[:], expr_sb[:, 1406:1407])
        nc.vector.tensor_add(g_col[:], g_col[:], gt_col[:])
        exprg_sb = wp.tile([12, HP], dt.float32)
        nc.vector.tensor_scalar_add(exprg_sb[:], expr_sb[:], g_col[:])

        # ---------------- epilogue: per 128-row tile ----------------
        def apply_def_tile(w_ap, verts_list, out_list):
            """verts/out: [128, 12] APs ((b,d) cols). Shared R per tile."""
            ang = ep_pool.tile([128, 12], dt.float32, tag="ang")
            nc.vector.tensor_scalar(ang[:], _sb(feat_t, [[1, 12]], 0), w_ap, None, op0=AluOp.mult)
            s_t = ep_pool.tile([128, 12], dt.float32, tag="sin")
            nc.scalar.activation(s_t[:], ang[:], ActFn.Sin, bias=0.0, scale=1.0)
            s2 = ep_pool.tile([128, 12], dt.float32, tag="sin2")
            nc.scalar.activation(s2[:], ang[:], ActFn.Sin, bias=0.0, scale=0.5)
            c_t = ep_pool.tile([128, 12], dt.float32, tag="cos")
            nc.vector.tensor_mul(c_t[:], s2[:], s2[:])
            nc.vector.tensor_scalar(c_t[:], c_t[:], -2.0, 1.0, op0=AluOp.mult, op1=AluOp.add)

            def col(tile_o, d):  # [128, 4] strided slice (b-major, comp d)
                return _sb(tile_o, [[3, 4]], d)

            sx, sy, sz = col(s_t, 0), col(s_t, 1), col(s_t, 2)
            cx, cy, cz = col(c_t, 0), col(c_t, 1), col(c_t, 2)
            R = ep_pool.tile([128, 36], dt.float32, tag="rot")

            def rc(d, j):  # [128, 4] strided output slice of R ((b,d,j) layout)
                return _sb(R, [[9, 4]], d * 3 + j)

            tmp2 = ep_pool.tile([128, 8], dt.float32, tag="rtmp")
            sxsy = _sb(tmp2, [[2, 4]], 0)
            cxsy = _sb(tmp2, [[2, 4]], 1)
            nc.vector.tensor_tensor(sxsy, sx, sy, op=AluOp.mult)
            nc.vector.tensor_tensor(cxsy, cx, sy, op=AluOp.mult)
            t4 = ep_pool.tile([128, 4], dt.float32, tag="t4")
            # row 0
            nc.vector.tensor_tensor(rc(0, 0), cy, cz, op=AluOp.mult)
            nc.vector.scalar_tensor_tensor(rc(0, 1), cy, -1.0, sz, op0=AluOp.mult, op1=AluOp.mult)
            nc.vector.tensor_copy(rc(0, 2), sy)
            # row 1
            nc.vector.tensor_tensor(t4[:], sxsy, cz, op=AluOp.mult)
            nc.vector.tensor_tensor(rc(1, 0), cx, sz, op=AluOp.mult)
            nc.vector.tensor_tensor(rc(1, 0), rc(1, 0), t4[:], op=AluOp.add)
            nc.vector.tensor_tensor(t4[:], sxsy, sz, op=AluOp.mult)
            nc.vector.tensor_tensor(rc(1, 1), cx, cz, op=AluOp.mult)
            nc.vector.tensor_tensor(rc(1, 1), rc(1, 1), t4[:], op=AluOp.subtract)
            nc.vector.scalar_tensor_tensor(rc(1, 2), sx, -1.0, cy, op0=AluOp.mult, op1=AluOp.mult)
            # row 2
            nc.vector.tensor_tensor(t4[:], cxsy, cz, op=AluOp.mult)
            nc.vector.tensor_tensor(rc(2, 0), sx, sz, op=AluOp.mult)
            nc.vector.tensor_tensor(rc(2, 0), rc(2, 0), t4[:], op=AluOp.subtract)
            nc.vector.tensor_tensor(t4[:], cxsy, sz, op=AluOp.mult)
            nc.vector.tensor_tensor(rc(2, 1), sx, cz, op=AluOp.mult)
            nc.vector.tensor_tensor(rc(2, 1), rc(2, 1), t4[:], op=AluOp.add)
            nc.vector.tensor_tensor(rc(2, 2), cx, cy, op=AluOp.mult)

            for verts_ap, out_ap in zip(verts_list, out_list):
                loc = ep_pool.tile([128, 12], dt.float32, tag="loc")
                nc.vector.tensor_tensor(loc[:], verts_ap, _sb(feat_t, [[1, 12]], 36), op=AluOp.subtract)
                nc.vector.tensor_tensor(loc[:], loc[:], _sb(feat_t, [[1, 12]], 24), op=AluOp.mult)
                acc = ep_pool.tile([128, 12], dt.float32, tag="acc12")
                mt = ep_pool.tile([128, 12], dt.float32, tag="mt12")
                for d in range(3):
                    in0 = _sb(loc, [[3, 4], [0, 3]], d)
                    in1 = _sb(R, [[9, 4], [1, 3]], d * 3)
                    if d == 0:
                        nc.vector.tensor_tensor(_sb(acc, [[3, 4], [1, 3]]), in0, in1, op=AluOp.mult)
                    else:
                        nc.vector.tensor_tensor(_sb(mt, [[3, 4], [1, 3]]), in0, in1, op=AluOp.mult)
                        nc.vector.tensor_tensor(acc[:], acc[:], mt[:], op=AluOp.add)
                # + trans*w + origin
                tw = ep_pool.tile([128, 12], dt.float32, tag="tw12")
                nc.vector.tensor_scalar(tw[:], _sb(feat_t, [[1, 12]], 12), w_ap, None, op0=AluOp.mult)
                nc.vector.tensor_tensor(acc[:], acc[:], tw[:], op=AluOp.add)
                nc.vector.tensor_tensor(out_ap, acc[:], _sb(feat_t, [[1, 12]], 36), op=AluOp.add)

        for t in range(NTILE):
            tdef_b = _sb(tdef_all, [[0, 4], [1, 3]], t * 3)  # [128, (b,jj)] broadcast over b
            ev_t = ep_pool.tile([128, 12], dt.float32, tag="evt")
            if t < NHT:
                pse = pp.tile([128, 512], dt.float32, tag="mm")
                nc.tensor.transpose(pse[:, 0:12], exprg_sb[:, t * 128:(t + 1) * 128], ident[0:12, 0:12])
                nc.vector.tensor_tensor(ev_t[:], pse[:, 0:12], tdef_b, op=AluOp.add)
            else:
                it = t - NHT
                nc.vector.tensor_tensor(ev_t[:], dicto_t[:, t, :], inn_all[:, it, :], op=AluOp.add)
                nc.vector.tensor_tensor(ev_t[:], ev_t[:], tdef_b, op=AluOp.add)
            sync.dma_start(_mk_ap(ev_o.ap(), t * 128 * 12, [[12, 128], [1, 12]]), ev_t[:])
            oi_in = ep_pool.tile([128, 12], dt.float32, tag="oit")
            nc.vector.tensor_tensor(oi_in[:], dicto_t[:, t, :], tdef_b, op=AluOp.add)
            df_t = ep_pool.tile([128, 12], dt.float32, tag="dft")
            oi_t = ep_pool.tile([128, 12], dt.float32, tag="oio")
            apply_def_tile(pw_all[:, t, :], [ev_t[:], oi_in[:]], [df_t[:], oi_t[:]])
            sync.dma_start(_mk_ap(df_o.ap(), t * 128 * 12, [[12, 128], [1, 12]]), df_t[:])
            sync.dma_start(_mk_ap(oi_o.ap(), t * 128 * 12, [[12, 128], [1, 12]]), oi_t[:])

    nc.compile()
    return nc


def _prep(inputs):
    """Host prep: returns (in_maps, meta)."""
    f32 = np.float32
    feats = np.asarray(inputs["features"], f32)
    canonical = np.asarray(inputs["canonical"], f32)
    expr_basis = np.asarray(inputs["expr_basis"], f32)
    rinv = np.asarray(inputs["rinv"], f32)
    corner_w = np.asarray(inputs["corner_w"], f32)
    L_inv = np.asarray(inputs["L_inv"], f32)
    origin = np.asarray(inputs["transform_origin"], f32)
    faces = np.asarray(inputs["faces"]).astype(np.int64)

    code = feats[:, :53]
    enc_coords = _embed(canonical)                                   # [V, 27]
    dict_ = canonical[None] + np.einsum("be,evd->bvd", code, expr_basis).astype(f32)

    # enc_pts
    fv = canonical[:HEAD][faces]                                     # [F,3,3]
    e1, e2 = fv[:, 1] - fv[:, 0], fv[:, 2] - fv[:, 0]
    n = np.cross(e1, e2)
    n = n / (np.linalg.norm(n, axis=-1, keepdims=True) + 1e-8)
    points = np.concatenate([fv.mean(1), n], -1).astype(f32)         # [F,6]
    enc_pts = _embed(points)                                         # [F,54]

    # ict_jac
    v0 = dict_[:, faces[:, 0]]
    v1 = dict_[:, faces[:, 1]]
    v2 = dict_[:, faces[:, 2]]
    a1, a2 = v1 - v0, v2 - v0
    nn = np.cross(a1, a2)
    nn = nn / (np.linalg.norm(nn, axis=-1, keepdims=True) + 1e-8)
    E = np.stack([a1, a2, nn], -2)                                   # [B,F,3,3]
    ict_jac = np.einsum("fij,bfjk->bfik", rinv, E).astype(f32)       # [B,F,3,3]

    m_row = L_inv[6705:9409].mean(0).astype(f32)                     # [HEAD]

    dict12 = dict_.transpose(1, 0, 2).reshape(V, 12).astype(f32)     # [v, (b,d)]

    # per-core inn chunk sizes
    inn_sizes = [ICH] * 7 + [VI - 7 * ICH]

    # rounds (max over cores per corner)
    occ_all = np.zeros((NCORES, 3, FCH), np.int64)
    for k in range(NCORES):
        fc = faces[k * FCH:(k + 1) * FCH]
        for c in range(3):
            occ_all[k, c] = _occurrence_rank(fc[:, c])
    rpc = tuple(int(occ_all[:, c].max()) + 1 for c in range(3))

    in_maps = []
    for k in range(NCORES):
        fc = faces[k * FCH:(k + 1) * FCH]                            # [2750, 3]
        # xe [116, 11264]
        xe = np.zeros((116, NED), f32)
        cols = np.arange(FCH) * 4
        for b in range(B):
            xe[0:54, cols + b] = enc_pts[k * FCH:(k + 1) * FCH].T
            xe[54:107, cols + b] = code[b][:, None]
            xe[107:116, cols + b] = ict_jac[b, k * FCH:(k + 1) * FCH].reshape(FCH, 9).T
        # icj [128, TF, 36]
        icj = np.zeros((128, TF, 36), f32)
        jr = ict_jac[:, k * FCH:(k + 1) * FCH].reshape(B, FCH, 9)     # [B, f, 9]
        jr = jr.transpose(1, 0, 2).reshape(FCH, 36)                  # [f, (b,i,jj)]
        fl = np.arange(FCH)
        icj[fl % 128, fl // 128] = jr
        # cwp [128, TF, 9] (c,i)
        cwp = np.zeros((128, TF, 9), f32)
        cwp[fl % 128, fl // 128] = corner_w[k * FCH:(k + 1) * FCH].reshape(FCH, 9)
        # sidx [128, NROUND, IW]
        sidx = np.zeros((128, sum(rpc), IW), np.int16)
        ridx = 0
        for c in range(3):
            for r in range(rpc[c]):
                tgt = np.full(FPAD, DUMP, np.int64)
                sel = occ_all[k, c] == r
                tgt[:FCH][sel] = fc[sel, c]
                sidx[:, ridx, :] = _wrap16(tgt)
                ridx += 1
        # lit [11264, 1408]
        lit = np.zeros((NRR, HP), f32)
        lit[:HEAD, :HCH] = L_inv[k * HCH:(k + 1) * HCH].T
        lit[:HEAD, 1406] = m_row
        # own rows
        ich = inn_sizes[k]
        head_rows = np.arange(k * HCH, (k + 1) * HCH)
        inn_rows = HEAD + k * ICH + np.arange(ich)
        xtd = np.zeros((27, OWNP), f32)
        xtd[:, 0:HCH] = enc_coords[head_rows].T
        xtd[:, HP:HP + ich] = enc_coords[inn_rows].T
        xpw = np.zeros((3, OWNP), f32)
        xpw[:, 0:HCH] = canonical[head_rows].T
        xpw[:, HP:HP + ich] = canonical[inn_rows].T
        # xin [83, 1536]: cols = vi_local*4 + b
        xin = np.zeros((83, NIN), f32)
        icols = np.arange(ich) * 4
        for b in range(B):
            xin[0:27, icols + b] = enc_coords[inn_rows].T
            xin[27:80, icols + b] = code[b][:, None]
            xin[80:83, icols + b] = dict_[b, inn_rows].T
        dicto = np.zeros((OWNP, 12), f32)
        dicto[0:HCH] = dict12[head_rows]
        dicto[HP:HP + ich] = dict12[inn_rows]
        dictb = np.zeros((2816, 12), f32)
        dictb[0:2704] = dict12[6705:9409]
        feat = np.zeros((128, 64), f32)
        eul = feats[:, 53:56].reshape(-1)        # (b,d)
        trn = feats[:, 56:59].reshape(-1)
        scl = np.repeat(feats[:, 59], 3)     # scale is [B,1], broadcast over d
        org = np.tile(origin, B)
        feat[:, 0:12] = eul
        feat[:, 12:24] = trn
        feat[:, 24:36] = scl
        feat[:, 36:48] = org
        im = dict(
            xe=xe, icj=icj, cwp=cwp, sidx=sidx, lit=lit,
            xtd=xtd, xpw=xpw, xin=xin, dicto=dicto, dictb=dictb,
            feat=feat, codet=np.ascontiguousarray(code.T),
            ew_in=inputs["ed_Win"], ew_h=inputs["ed_Wh"], ew_out=inputs["ed_Wout"],
            eb_in=inputs["ed_bin"], eb_h=inputs["ed_bh"], eb_out=inputs["ed_bout"],
            tw_in=inputs["td_Win"], tw_h=inputs["td_Wh"], tw_out=inputs["td_Wout"],
            tb_in=inputs["td_bin"], tb_h=inputs["td_bh"], tb_out=inputs["td_bout"],
            iw_in=inputs["inn_Win"], iw_h=inputs["inn_Wh"], iw_out=inputs["inn_Wout"],
            ib_in=inputs["inn_bin"], ib_h=inputs["inn_bh"], ib_out=inputs["inn_bout"],
            gw_in=inputs["gt_Win"], gw_h=inputs["gt_Wh"][0], gw_out=inputs["gt_Wout"],
            gb_in=inputs["gt_bin"], gb_h=inputs["gt_bh"][0], gb_out=inputs["gt_bout"],
            pw_in=inputs["pw_Win"], pw_h=inputs["pw_Wh"][0], pw_out=inputs["pw_Wout"],
            pb_in=inputs["pw_bin"], pb_h=inputs["pw_bh"][0], pb_out=inputs["pw_bout"],
            scal=np.zeros((128, 16), f32),
        )
        im = {kk: np.ascontiguousarray(np.asarray(vv, f32)) if np.asarray(vv).dtype != np.int16
              else vv for kk, vv in im.items()}
        in_maps.append(im)

    scales = tuple(
        tuple(float(-1.0 / (2.0 * a * a)) for a in np.asarray(inputs[p + "_a"], f32))
        for p in ("ed", "td", "inn", "gt", "pw")
    )
    meta = dict(rpc=rpc, inn_sizes=inn_sizes, scales=scales,
                template=np.asarray(inputs["template"], f32),
                features=np.asarray(inputs["features"], f32))
    return in_maps, meta


def _assemble(results, meta):
    inn_sizes = meta["inn_sizes"]
    tdef = np.zeros((V, 3), np.float32)
    pw = np.zeros((V, 1), np.float32)
    ev = np.zeros((V, 12), np.float32)
    df = np.zeros((V, 12), np.float32)
    oi = np.zeros((V, 12), np.float32)
    for k in range(NCORES):
        r = results[k]
        ich = inn_sizes[k]
        hr = slice(k * HCH, (k + 1) * HCH)
        ir = slice(HEAD + k * ICH, HEAD + k * ICH + ich)
        tdef[hr] = r["tdef_o"][0:HCH]
        tdef[ir] = r["tdef_o"][HP:HP + ich]
        pw[hr] = r["pw_o"][0:HCH]
        pw[ir] = r["pw_o"][HP:HP + ich]
        for dst, nm in ((ev, "ev_o"), (df, "df_o"), (oi, "oi_o")):
            dst[hr] = r[nm][0:HCH]
            dst[ir] = r[nm][HP:HP + ich]
    def to_bvd(x):
        return np.ascontiguousarray(x.reshape(V, B, 3).transpose(1, 0, 2))
    template_mesh = meta["template"] + tdef
    return (meta["features"], template_mesh, to_bvd(ev), pw, to_bvd(df), to_bvd(oi))


def kernel(**inputs):
    in_maps, meta = _prep(inputs)
    key = (meta["rpc"], meta["scales"])
    if key not in _CACHE:
        _build.scales = meta["scales"]
        _CACHE[key] = _build(meta["rpc"])
    nc = _CACHE[key]
    from concourse.bass_utils import run_bass_kernel_spmd
    res = run_bass_kernel_spmd(nc, in_maps, core_ids=list(range(NCORES)))
    return _assemble(res.results, meta)
